# revision 12
# baseline (speedup 1.0000x reference)
"""Trainium2 Bass kernel for nn_MultiModalSplitNorm (static grouped GEMM / MoE).

Problem: x [16384, 4096] f32, W [4, 4096, 4096] bf16, group_sizes = [4096]*4.
Output: y[t] = x[t] @ W[g(t)].T  (bf16 matmul, f32 accumulate/output).

Sharding (8 cores): expert-parallel x output-column-parallel.
Core c handles expert g = c//2, output columns half h = c%2:
    y[g*4096:(g+1)*4096, h*2048:(h+1)*2048] =
        x[g*4096:(g+1)*4096] @ W[g, h*2048:(h+1)*2048, :].T

Host-side sharding ships both operands in the layout the PE consumes
(layout-only transforms; all arithmetic, including the bf16 cast of x,
happens on device):
  - w: [HIDDEN, O_HALF] = W_half.T           (contiguous weight stream)
  - x: [TB, HIDDEN, 128] t-block-tiled x.T   (one contiguous 2 MB read per
                                              128-token block)

Per-core kernel (T=4096 tokens, K=4096 contraction, O=2048 outputs):
  - W^T streamed once on the scalar HWDGE queue, resident in SBUF
    (128 KB/partition).
  - per token block: one 2 MB DMA (sync queue) -> DVE cast f32->bf16 ->
    lhsT tiles [128, 128].
  - PE: per K-block ldweights(x^T tile) + 4 matmuls (N=512) accumulating
    into 4 PSUM banks; even/odd token blocks use disjoint bank groups
    (double buffering). The first two token blocks are interleaved K-major
    so PE weight-tile consumption (1.7 us/tile) matches the W stream
    arrival rate (~1.5 us/tile) instead of stalling.
  - PSUM -> SBUF copy (f32) -> DMA out.

No DMA-transpose instructions anywhere: transpose<->copy transitions
serialize the whole DMA subsystem (HW hang workaround) and were measured
to throttle the weight stream ~2.4x during the prologue.
"""

import os
import sys

import numpy as np

# ---- constants (hardcoded per spec; kernel.py must be self-contained) ----
NUM_EXPERTS = 4
GROUP = 4096  # tokens per expert
HIDDEN = 4096  # contraction dim
TOTAL = NUM_EXPERTS * GROUP
N_CORES = 8
O_HALF = HIDDEN // 2  # 2048 output columns per core

P = 128
IB = HIDDEN // P  # 32 k-blocks
NB = 512  # matmul moving free dim (one PSUM bank)
OB = O_HALF // NB  # 4 psum banks per token block


def _ensure_paths():
    for p in ("/opt/trn_rl_repo", "/root/.axon_site", "/root/.axon_site/_ro/pypackages"):
        if os.path.isdir(p) and p not in sys.path:
            sys.path.append(p)
    try:
        import concourse  # noqa: F401
    except ImportError:
        raise RuntimeError("concourse not importable; check PYTHONPATH")


_NC_CACHE = {}


def build_nc(tb_count=GROUP // P):
    """Build + compile the per-core Bass program. tb_count = 128-token blocks."""
    if tb_count in _NC_CACHE:
        return _NC_CACHE[tb_count]
    _ensure_paths()
    import concourse.mybir as mybir
    import concourse.tile as tile
    from concourse import bacc

    assert tb_count % 2 == 0
    n_pairs = tb_count // 2
    U = 2 * P  # tokens per pair slab
    # ib-chunk sizes per pair load: small first chunk so the first matmuls
    # can start as early as possible (prologue latency)
    CHUNKS = (2, 6, 12, 12)
    assert sum(CHUNKS) == IB

    nc = bacc.Bacc("TRN2", target_bir_lowering=False, debug=False)
    x_d = nc.dram_tensor(
        "x", [n_pairs, HIDDEN, U], mybir.dt.float32, kind="ExternalInput"
    )
    w_d = nc.dram_tensor("w", [HIDDEN, O_HALF], mybir.dt.bfloat16, kind="ExternalInput")
    y_d = nc.dram_tensor("y", [tb_count * P, O_HALF], mybir.dt.float32, kind="ExternalOutput")
    x_ap, w_ap, y_ap = x_d.ap(), w_d.ap(), y_d.ap()

    with tile.TileContext(nc) as tc:
        from contextlib import ExitStack

        with ExitStack() as ctx:
            wt_pool = ctx.enter_context(tc.tile_pool(name="wt", bufs=1))
            xf_pool = ctx.enter_context(tc.tile_pool(name="xf", bufs=2))
            xb_pool = ctx.enter_context(tc.tile_pool(name="xb", bufs=2))
            out_pool = ctx.enter_context(tc.tile_pool(name="out", bufs=2))
            psum_pool = ctx.enter_context(
                tc.tile_pool(name="psum", bufs=1, space="PSUM")
            )

            # ---- W^T resident: 32 tiles [128, O_HALF] bf16, scalar queue ----
            wT = []
            for ib in range(IB):
                t = wt_pool.tile(
                    [P, O_HALF], mybir.dt.bfloat16, name=f"wT{ib}", tag=f"wT{ib}"
                )
                nc.scalar.dma_start(t[:], w_ap[ib * P : (ib + 1) * P, :])
                wT.append(t)

            def load_cast(pr):
                """DMA one pair slab (ib-chunked for early start), cast bf16.

                xf is a transient chunk-sized staging tile: DMA f32 chunk in,
                cast to its slice of the pair-resident xb, release."""
                src = x_ap[pr].rearrange("(ib p) u -> p ib u", p=P)
                xb = xb_pool.tile(
                    [P, IB, U], mybir.dt.bfloat16, name=f"xb_{pr}", tag="xb"
                )
                ib0 = 0
                for c, ch in enumerate(CHUNKS):
                    s = slice(ib0, ib0 + ch)
                    ib0 += ch
                    xf = xf_pool.tile(
                        [P, max(CHUNKS), U],
                        mybir.dt.float32,
                        name=f"xf_{pr}_{c}",
                        tag="xf",
                    )
                    nc.sync.dma_start(xf[:, :ch, :], src[:, s, :])
                    nc.vector.tensor_copy(xb[:, s, :], xf[:, :ch, :])
                return xb

            def alloc_psum(tb):
                grp = (tb % 2) * OB  # even tb -> banks 0-3, odd tb -> banks 4-7
                return [
                    psum_pool.tile(
                        [P, NB],
                        mybir.dt.float32,
                        name=f"ps_{tb}_{ob}",
                        tag=f"bank{grp + ob}",
                    )
                    for ob in range(OB)
                ]

            def mm_group(ps, xb, ib, t):
                lhsT = xb[:, ib, t * P : (t + 1) * P]
                for ob in range(OB):
                    nc.tensor.matmul(
                        ps[ob][:],
                        lhsT,
                        wT[ib][:, ob * NB : (ob + 1) * NB],
                        start=(ib == 0),
                        stop=(ib == IB - 1),
                    )

            def evac_store(tb, ps):
                yo = out_pool.tile(
                    [P, O_HALF], mybir.dt.float32, name=f"yo_{tb}", tag="yo"
                )
                for ob in range(OB):
                    nc.any.tensor_copy(out=yo[:, ob * NB : (ob + 1) * NB], in_=ps[ob][:])
                nc.sync.dma_start(y_ap[tb * P : (tb + 1) * P, :], yo[:])

            for pr in range(n_pairs):
                xb = load_cast(pr)
                ps0 = alloc_psum(2 * pr)
                ps1 = alloc_psum(2 * pr + 1)
                if pr == 0:
                    # K-major interleave of both token blocks: PE consumes one
                    # wT tile per 8 matmuls (~1.7 us), matching the W stream
                    # arrival rate so the prologue doesn't stall on weights.
                    for ib in range(IB):
                        mm_group(ps0, xb, ib, 0)
                        mm_group(ps1, xb, ib, 1)
                else:
                    for ib in range(IB):
                        mm_group(ps0, xb, ib, 0)
                    for ib in range(IB):
                        mm_group(ps1, xb, ib, 1)
                evac_store(2 * pr, ps0)
                evac_store(2 * pr + 1, ps1)

    nc.compile()
    _NC_CACHE[tb_count] = nc
    return nc


def _shard_inputs(x, W):
    import ml_dtypes

    x = np.asarray(x)
    if x.dtype != np.float32:
        x = x.astype(np.float32)
    W = np.asarray(W)
    if W.dtype != ml_dtypes.bfloat16:
        W = W.astype(ml_dtypes.bfloat16)
    n_pairs = GROUP // (2 * P)
    in_maps = []
    for c in range(N_CORES):
        g, h = c // 2, c % 2
        xg = x[g * GROUP : (g + 1) * GROUP]
        # pair-slab-tiled transpose: [n_pairs, HIDDEN, 256], element
        # (pr, i, u) = x[g*GROUP + pr*256 + u, i]  (layout-only; values
        # unchanged; 1 KB contiguous partition lines for DMA efficiency)
        xt = np.ascontiguousarray(xg.reshape(n_pairs, 2 * P, HIDDEN).transpose(0, 2, 1))
        in_maps.append(
            {
                "x": xt,
                # weight shard shipped transposed: [HIDDEN, O_HALF]
                "w": np.ascontiguousarray(W[g, h * O_HALF : (h + 1) * O_HALF, :].T),
            }
        )
    return in_maps


def kernel(x, W, group_sizes=None, **_ignored):
    if group_sizes is not None:
        gs = np.asarray(group_sizes).astype(np.int64)
        assert gs.shape == (NUM_EXPERTS,) and np.all(gs == GROUP), (
            f"kernel compiled for static group_sizes=[{GROUP}]*{NUM_EXPERTS}, got {gs}"
        )
    _ensure_paths()
    from concourse.bass_utils import run_bass_kernel_spmd

    nc = build_nc()
    in_maps = _shard_inputs(x, W)
    res = run_bass_kernel_spmd(nc, in_maps, core_ids=list(range(N_CORES)))
    y = np.empty((TOTAL, HIDDEN), dtype=np.float32)
    for c in range(N_CORES):
        g, h = c // 2, c % 2
        y[g * GROUP : (g + 1) * GROUP, h * O_HALF : (h + 1) * O_HALF] = res.results[c][
            "y"
        ]
    return y


# revision 28
# speedup vs baseline: 1.0158x; 1.0158x over previous
"""Trainium2 Bass kernel for nn_MultiModalSplitNorm (static grouped GEMM / MoE).

Problem: x [16384, 4096] f32, W [4, 4096, 4096] bf16, group_sizes = [4096]*4.
Output: y[t] = x[t] @ W[g(t)].T  (bf16 matmul, f32 accumulate/output).

Sharding (8 cores): expert-parallel x output-column-parallel.
Core c handles expert g = c//2, output columns half h = c%2:
    y[g*4096:(g+1)*4096, h*2048:(h+1)*2048] =
        x[g*4096:(g+1)*4096] @ W[g, h*2048:(h+1)*2048, :].T

Host-side sharding ships both operands in the layout the PE consumes
(layout-only transforms; all arithmetic, including the bf16 cast of x,
happens on device):
  - w: [HIDDEN, O_HALF] = W_half.T           (contiguous weight stream)
  - x: [TB, HIDDEN, 128] t-block-tiled x.T   (one contiguous 2 MB read per
                                              128-token block)

Per-core kernel (T=4096 tokens, K=4096 contraction, O=2048 outputs):
  - W^T streamed once on the scalar HWDGE queue, resident in SBUF
    (128 KB/partition).
  - per token block: one 2 MB DMA (sync queue) -> DVE cast f32->bf16 ->
    lhsT tiles [128, 128].
  - PE: per K-block ldweights(x^T tile) + 4 matmuls (N=512) accumulating
    into 4 PSUM banks; even/odd token blocks use disjoint bank groups
    (double buffering). The first two token blocks are interleaved K-major
    so PE weight-tile consumption (1.7 us/tile) matches the W stream
    arrival rate (~1.5 us/tile) instead of stalling.
  - PSUM -> SBUF copy (f32) -> DMA out.

No DMA-transpose instructions anywhere: transpose<->copy transitions
serialize the whole DMA subsystem (HW hang workaround) and were measured
to throttle the weight stream ~2.4x during the prologue.
"""

import os
import sys

import numpy as np

# ---- constants (hardcoded per spec; kernel.py must be self-contained) ----
NUM_EXPERTS = 4
GROUP = 4096  # tokens per expert
HIDDEN = 4096  # contraction dim
TOTAL = NUM_EXPERTS * GROUP
N_CORES = 8
O_HALF = HIDDEN // 2  # 2048 output columns per core

P = 128
IB = HIDDEN // P  # 32 k-blocks
NB = 512  # matmul moving free dim (one PSUM bank)
OB = O_HALF // NB  # 4 psum banks per token block


def _ensure_paths():
    for p in ("/opt/trn_rl_repo", "/root/.axon_site", "/root/.axon_site/_ro/pypackages"):
        if os.path.isdir(p) and p not in sys.path:
            sys.path.append(p)
    try:
        import concourse  # noqa: F401
    except ImportError:
        raise RuntimeError("concourse not importable; check PYTHONPATH")


_NC_CACHE = {}


def build_nc(tb_count=GROUP // P):
    """Build + compile the per-core Bass program. tb_count = 128-token blocks."""
    if tb_count in _NC_CACHE:
        return _NC_CACHE[tb_count]
    _ensure_paths()
    import concourse.mybir as mybir
    import concourse.tile as tile
    from concourse import bacc

    assert tb_count % 2 == 0
    n_pairs = tb_count // 2
    U = 2 * P  # tokens per pair slab
    # ib-chunk sizes per pair load: small first chunk so the first matmuls
    # can start as early as possible (prologue latency)
    CHUNKS = (2, 6, 12, 12)
    assert sum(CHUNKS) == IB

    nc = bacc.Bacc("TRN2", target_bir_lowering=False, debug=False)
    x_d = nc.dram_tensor(
        "x", [n_pairs, HIDDEN, U], mybir.dt.float32, kind="ExternalInput"
    )
    w_d = nc.dram_tensor("w", [HIDDEN, O_HALF], mybir.dt.bfloat16, kind="ExternalInput")
    y_d = nc.dram_tensor("y", [tb_count * P, O_HALF], mybir.dt.float32, kind="ExternalOutput")
    x_ap, w_ap, y_ap = x_d.ap(), w_d.ap(), y_d.ap()

    with tile.TileContext(nc) as tc:
        from contextlib import ExitStack

        with ExitStack() as ctx:
            wt_pool = ctx.enter_context(tc.tile_pool(name="wt", bufs=1))
            xf_pool = ctx.enter_context(tc.tile_pool(name="xf", bufs=2))
            xb_pool = ctx.enter_context(tc.tile_pool(name="xb", bufs=2))
            out_pool = ctx.enter_context(tc.tile_pool(name="out", bufs=2))
            psum_pool = ctx.enter_context(
                tc.tile_pool(name="psum", bufs=1, space="PSUM")
            )

            # ---- W^T resident: 32 tiles [128, O_HALF] bf16, scalar queue ----
            wT = []
            for ib in range(IB):
                t = wt_pool.tile(
                    [P, O_HALF], mybir.dt.bfloat16, name=f"wT{ib}", tag=f"wT{ib}"
                )
                nc.scalar.dma_start(t[:], w_ap[ib * P : (ib + 1) * P, :])
                wT.append(t)

            def load_cast(pr):
                """DMA one pair slab (ib-chunked for early start), cast bf16.

                xf is a transient chunk-sized staging tile: DMA f32 chunk in,
                cast to its slice of the pair-resident xb, release."""
                src = x_ap[pr].rearrange("(ib p) u -> p ib u", p=P)
                xb = xb_pool.tile(
                    [P, IB, U], mybir.dt.bfloat16, name=f"xb_{pr}", tag="xb"
                )
                # pair 1 loads via the scalar queue, FIFO-behind the W bulk:
                # during the HBM-bound prologue the W stream and pair-0 x get
                # the bandwidth, and pair-1 prefetch starts right as W ends.
                eng = nc.scalar if pr == 1 else nc.sync
                ib0 = 0
                for c, ch in enumerate(CHUNKS):
                    s = slice(ib0, ib0 + ch)
                    ib0 += ch
                    xf = xf_pool.tile(
                        [P, max(CHUNKS), U],
                        mybir.dt.float32,
                        name=f"xf_{pr}_{c}",
                        tag="xf",
                    )
                    eng.dma_start(xf[:, :ch, :], src[:, s, :])
                    nc.vector.tensor_copy(xb[:, s, :], xf[:, :ch, :])
                return xb

            def alloc_psum(tb):
                grp = (tb % 2) * OB  # even tb -> banks 0-3, odd tb -> banks 4-7
                return [
                    psum_pool.tile(
                        [P, NB],
                        mybir.dt.float32,
                        name=f"ps_{tb}_{ob}",
                        tag=f"bank{grp + ob}",
                    )
                    for ob in range(OB)
                ]

            def mm_group(ps, xb, ib, t):
                lhsT = xb[:, ib, t * P : (t + 1) * P]
                for ob in range(OB):
                    nc.tensor.matmul(
                        ps[ob][:],
                        lhsT,
                        wT[ib][:, ob * NB : (ob + 1) * NB],
                        start=(ib == 0),
                        stop=(ib == IB - 1),
                    )

            def evac_store(tb, ps):
                yo = out_pool.tile(
                    [P, O_HALF], mybir.dt.float32, name=f"yo_{tb}", tag="yo"
                )
                for ob in range(OB):
                    # explicit ACT copy: keep DVE free for the x casts
                    nc.scalar.copy(out=yo[:, ob * NB : (ob + 1) * NB], in_=ps[ob][:])
                nc.sync.dma_start(y_ap[tb * P : (tb + 1) * P, :], yo[:])

            for pr in range(n_pairs):
                xb = load_cast(pr)
                ps0 = alloc_psum(2 * pr)
                ps1 = alloc_psum(2 * pr + 1)
                if pr == 0:
                    # K-major interleave of both token blocks: PE consumes one
                    # wT tile per 8 matmuls (~1.7 us), matching the W stream
                    # arrival rate so the prologue doesn't stall on weights.
                    for ib in range(IB):
                        mm_group(ps0, xb, ib, 0)
                        mm_group(ps1, xb, ib, 1)
                else:
                    for ib in range(IB):
                        mm_group(ps0, xb, ib, 0)
                    for ib in range(IB):
                        mm_group(ps1, xb, ib, 1)
                evac_store(2 * pr, ps0)
                evac_store(2 * pr + 1, ps1)

    nc.compile()
    _NC_CACHE[tb_count] = nc
    return nc


def _shard_inputs(x, W):
    import ml_dtypes

    x = np.asarray(x)
    if x.dtype != np.float32:
        x = x.astype(np.float32)
    W = np.asarray(W)
    if W.dtype != ml_dtypes.bfloat16:
        W = W.astype(ml_dtypes.bfloat16)
    n_pairs = GROUP // (2 * P)
    in_maps = []
    for c in range(N_CORES):
        g, h = c // 2, c % 2
        xg = x[g * GROUP : (g + 1) * GROUP]
        # pair-slab-tiled transpose: [n_pairs, HIDDEN, 256], element
        # (pr, i, u) = x[g*GROUP + pr*256 + u, i]  (layout-only; values
        # unchanged; 1 KB contiguous partition lines for DMA efficiency)
        xt = np.ascontiguousarray(xg.reshape(n_pairs, 2 * P, HIDDEN).transpose(0, 2, 1))
        in_maps.append(
            {
                "x": xt,
                # weight shard shipped transposed: [HIDDEN, O_HALF]
                "w": np.ascontiguousarray(W[g, h * O_HALF : (h + 1) * O_HALF, :].T),
            }
        )
    return in_maps


def kernel(x, W, group_sizes=None, **_ignored):
    if group_sizes is not None:
        gs = np.asarray(group_sizes).astype(np.int64)
        assert gs.shape == (NUM_EXPERTS,) and np.all(gs == GROUP), (
            f"kernel compiled for static group_sizes=[{GROUP}]*{NUM_EXPERTS}, got {gs}"
        )
    _ensure_paths()
    from concourse.bass_utils import run_bass_kernel_spmd

    nc = build_nc()
    in_maps = _shard_inputs(x, W)
    res = run_bass_kernel_spmd(nc, in_maps, core_ids=list(range(N_CORES)))
    y = np.empty((TOTAL, HIDDEN), dtype=np.float32)
    for c in range(N_CORES):
        g, h = c // 2, c % 2
        y[g * GROUP : (g + 1) * GROUP, h * O_HALF : (h + 1) * O_HALF] = res.results[c][
            "y"
        ]
    return y
